# revision 81
# baseline (speedup 1.0000x reference)
"""Trainium2 Bass kernel for a dense transformer block (pre-LN, causal attn, FFN).

Sharding: pure data-parallel over batch. B=128 is split into 8 slices of 16;
each NeuronCore runs the full block on its slice with replicated weights.
No collectives.

v2 layout strategy (vs v1):
  - all matmul operands bf16 (fp32 PSUM accumulate): PE transposes at
    1 cyc/row, no f32r N>=256 width constraint, half the weight DMA/SBUF
  - batch elements processed in PAIRS: weight-side matmuls (QKV q/k, FFN1)
    take a 512-wide moving dim covering both elements -> half the
    instruction count at the same PE rows
  - causal-aware attention tiles: the fully-masked (t2-block1 x t1-block0)
    quarter of S/exp/PV is never computed; no mask add - exp runs on raw
    scores and the two diagonal staircases are zeroed afterwards with two
    gpsimd affine_selects over all 6 heads at once
  - V tiles carry a ones-column per head ([128, tt, 6*65]) so the softmax
    row-sums fall out of the PV matmul for free (extra out partition row)
  - softmax 1/sum via reciprocal_approx_fast (single DVE op, ~51 ULP)
    reading the PSUM sum row directly, then PE ones-broadcast to [64, 512]
  - engine balance: LN normalize on DVE, staircase zero + bias adds on
    Pool, relu/copies mostly on ACT, psum adds on DVE
"""
import sys

sys.path.insert(0, "/opt/trn_rl_repo")

import numpy as np

B, T, C, H, D = 128, 256, 384, 6, 64
NCORES = 8
BL = B // NCORES  # 16 batch elements per core
EPS = 1e-5
CT = C // 128      # 3 c-tiles
FT = 4 * C // 128  # 12 f-tiles
TT = T // 128      # 2 t-tiles
SC = D ** -0.5

_cache = {}


def build(reps=1, nb=BL, dbg=False, loop_reps=1, skip=(), tr_f32=False,
          pv_clean=False, ffn8=False):
    import concourse.bacc as bacc
    import concourse.bass as bass
    import concourse.mybir as mybir
    import concourse.tile as tile
    from concourse.masks import make_identity

    # Pin all activations to the one table that serves every function we
    # use (ln, exp, identity, copy, relu) so the kernel pays exactly one
    # ACT table load instead of thrashing 1.28us reloads.
    if not hasattr(bacc, "_orig_get_activation_tables"):
        bacc._orig_get_activation_tables = bacc.get_activation_tables

        def _pinned_tables(arch):
            t = bacc._orig_get_activation_tables(arch)
            keep = "natural_log_exp_and_others"
            assert keep in t
            return {k: (v if k == keep else set()) for k, v in t.items()}

        bacc.get_activation_tables = _pinned_tables

    F32R, F32 = mybir.dt.float32r, mybir.dt.float32
    BF16 = mybir.dt.bfloat16
    FP8 = mybir.dt.float8e4
    DR = mybir.MatmulPerfMode.DoubleRow
    AF = mybir.ActivationFunctionType
    ALU = mybir.AluOpType

    assert nb % 2 == 0
    NP = nb // 2

    nc = bacc.Bacc("TRN2", target_bir_lowering=False, debug=False)

    xd = nc.dram_tensor("x", [nb, T, C], F32, kind="ExternalInput")
    wqkv = nc.dram_tensor("wqkv", [CT, 128, 3 * C], BF16, kind="ExternalInput")
    wproj = nc.dram_tensor("wproj", [CT, 128, C], BF16, kind="ExternalInput")
    if ffn8:
        w1 = nc.dram_tensor("w1", [2, 2, 128, 4 * C], FP8,
                            kind="ExternalInput")
        w2 = nc.dram_tensor("w2", [FT // 2, 2, 128, C], FP8,
                            kind="ExternalInput")
    else:
        w1 = nc.dram_tensor("w1", [CT, 128, 4 * C], BF16,
                            kind="ExternalInput")
        w2 = nc.dram_tensor("w2", [FT, 128, C], BF16, kind="ExternalInput")
    b1d = nc.dram_tensor("b1", [FT, 128], F32, kind="ExternalInput")
    qkbd = nc.dram_tensor("qkb", [2 * CT, 128], F32, kind="ExternalInput")
    vecd = nc.dram_tensor("vecs", [2, 128, C], F32, kind="ExternalInput")
    onesd = nc.dram_tensor("ones", [128, 64], F32R, kind="ExternalInput")
    outd = nc.dram_tensor("out", [nb, T, C], F32, kind="ExternalOutput")

    with tile.TileContext(nc) as tc:
        with tc.tile_pool(name="const", bufs=1) as cp, \
             tc.tile_pool(name="prx", bufs=6) as xp, \
             tc.tile_pool(name="prh", bufs=4) as hp_, \
             tc.tile_pool(name="pair", bufs=2) as pp, \
             tc.tile_pool(name="elem", bufs=3) as ep, \
             tc.tile_pool(name="vpool", bufs=4) as vp, \
             tc.tile_pool(name="x1p", bufs=5) as x1p, \
             tc.tile_pool(name="small", bufs=6) as sp, \
             tc.tile_pool(name="recp", bufs=2) as rp, \
             tc.tile_pool(name="ps", bufs=8, space="PSUM") as ps:

            # ---- constants ----
            wqkv_sb = cp.tile([128, CT, 3 * C], BF16)
            nc.sync.dma_start(out=wqkv_sb, in_=wqkv.rearrange("k p n -> p k n"))
            wproj_sb = cp.tile([128, CT, C], BF16)
            nc.sync.dma_start(out=wproj_sb, in_=wproj.rearrange("k p n -> p k n"))
            if ffn8:
                w1_sb = cp.tile([128, 2, 2, 4 * C], FP8)
                nc.sync.dma_start(out=w1_sb,
                                  in_=w1.rearrange("g i p n -> p g i n"))
                w2_sb = cp.tile([128, FT // 2, 2, C], FP8)
                nc.sync.dma_start(out=w2_sb,
                                  in_=w2.rearrange("j i p n -> p j i n"))
            else:
                w1_sb = cp.tile([128, CT, 4 * C], BF16)
                nc.sync.dma_start(out=w1_sb,
                                  in_=w1.rearrange("k p n -> p k n"))
                w2_sb = cp.tile([128, FT, C], BF16)
                nc.sync.dma_start(out=w2_sb,
                                  in_=w2.rearrange("k p n -> p k n"))
            b1_sb = cp.tile([128, FT], F32)
            nc.sync.dma_start(out=b1_sb, in_=b1d.rearrange("k p -> p k"))
            qkb_sb = cp.tile([128, 2 * CT], F32)
            nc.sync.dma_start(out=qkb_sb, in_=qkbd.rearrange("k p -> p k"))
            vec_sb = cp.tile([128, 2, C], F32)  # bproj_eff, b2 bcast
            nc.sync.dma_start(out=vec_sb, in_=vecd.rearrange("k p n -> p k n"))

            ident = cp.tile([128, 128], F32 if tr_f32 else BF16)
            make_identity(nc, ident)
            TRD = F32 if tr_f32 else BF16
            ones_sb = cp.tile([128, 64], F32R)
            nc.sync.dma_start(out=ones_sb, in_=onesd[:, :])
            ones_bf = cp.tile([128, 1], BF16)
            nc.scalar.copy(out=ones_bf, in_=ones_sb[:, 0:1])
            # bf16 0/1 staircase keep-mask (1 where t1 >= t2): built in f32
            # (gpsimd affine_select is only HW-proven on f32), converted once
            mask_f = cp.tile([128, 128], F32)
            nc.gpsimd.memset(mask_f, 1.0)
            nc.gpsimd.affine_select(
                out=mask_f, in_=mask_f, compare_op=ALU.is_ge, fill=0.0,
                base=0, pattern=[[1, 128]], channel_multiplier=-1)
            mask_bf = cp.tile([128, 2, 128], BF16)
            nc.scalar.copy(out=mask_bf[:, 0, :], in_=mask_f)
            nc.scalar.copy(out=mask_bf[:, 1, :], in_=mask_f)
            eps_sb = cp.tile([128, 1], F32)
            nc.vector.memset(eps_sb, EPS)

            BPROJ, B2 = range(2)

            def s_ln(src, dst, st, key):
                """dst(bf16) = (src - mean) * rsqrt(var + eps); token-major.
                gains/biases are folded into the downstream weights."""
                mv = sp.tile([128, TT, 2], F32, tag="mv")
                rs = sp.tile([128, TT], F32, tag="rs")
                for tt in range(TT):
                    stt = sp.tile([128, 6], F32, tag="st")
                    nc.vector.bn_stats(out=stt, in_=src[:, tt, :])
                    nc.vector.bn_aggr(out=mv[:, tt, :], in_=stt)
                # rstd = exp(-0.5*ln(var+eps)) for both tt in one op pair
                nc.scalar.activation(out=rs, in_=mv[:, :, 1], func=AF.Ln,
                                     bias=eps_sb, scale=1.0)
                nc.scalar.activation(out=rs, in_=rs, func=AF.Exp, scale=-0.5)
                for tt in range(TT):
                    nc.vector.tensor_scalar(
                        out=dst[:, tt, :], in0=src[:, tt, :],
                        scalar1=mv[:, tt, 0:1], scalar2=rs[:, tt:tt + 1],
                        op0=ALU.subtract, op1=ALU.mult)
                st[key] = dst

            def s_ln1(b, st):
                """x load + LN1 - hoisted early."""
                xb = xd[b].rearrange("(tt p) c -> p tt c", p=128)
                x_sb = xp.tile([128, TT, C], F32, tag="x")
                nc.sync.dma_start(out=x_sb, in_=xb)
                st["x"] = x_sb
                h = hp_.tile([128, TT, C], TRD, tag="h")
                s_ln(x_sb, h, st, "h")

            def transpose_pair(srcs, dst):
                """srcs: two [128, TT, C] bf16 token-major tiles ->
                dst [128, CT, 2, T] bf16 feature-major pair tile."""
                for e, src in enumerate(srcs):
                    tpa = ps.tile([128, 4, 128], TRD, tag="ps")
                    for i, (ct, tt) in enumerate(
                            [(0, 0), (0, 1), (1, 0), (1, 1)]):
                        nc.tensor.transpose(
                            tpa[:, i, :], src[:, tt, ct * 128:(ct + 1) * 128],
                            ident)
                    # dst free layout (ct, e, t): strides (2T, T, 1)
                    nc.vector.tensor_copy(
                        out=dst[:, 0:2, e, :].rearrange(
                            "p ct (tt f) -> p ct tt f", tt=TT),
                        in_=tpa.rearrange("p (ct tt) f -> p ct tt f", ct=2))
                    tpb = ps.tile([128, 2, 128], TRD, tag="ps")
                    for tt in range(TT):
                        nc.tensor.transpose(
                            tpb[:, tt, :], src[:, tt, 2 * 128:3 * 128], ident)
                    nc.scalar.copy(out=dst[:, 2, e, :], in_=tpb)

            def s_tr1(p, st):
                hT = pp.tile([128, CT, 2, T], BF16, tag="hT")
                transpose_pair((st["h0"], st["h1"]), hT)
                st["hT"] = hT

            def s_qkv(p, st):
                """paired q (feature-major bf16) + zero-padded per-head K
                tiles (so S matmuls contract over the full 128 partitions:
                sub-128 matmul tiles crash the runtime) + per-elem v
                (token-major bf16, ones-augmented per head for free sums)."""
                hT = st["hT"]
                qk = pp.tile([128, CT, 2, T], BF16, tag="qk")
                # kp layout [p, parity, hp, e, t2blk, 128]: head h = band
                # po:po+64 of (h%2, h//2); the opposite 64-partition half of
                # each band is zero so q's other head contracts to 0.
                kp = pp.tile([128, 2, CT, 2, 2, 128], BF16, tag="kp")
                st["qk"], st["kp"] = qk, kp
                nc.gpsimd.memset(kp[0:64, 1], 0.0)
                nc.gpsimd.memset(kp[64:128, 0], 0.0)
                for jt in range(CT):  # q c-tiles
                    acc = ps.tile([128, 2, T], F32, tag="ps")
                    for kt in range(CT):
                        nc.tensor.matmul(
                            acc,
                            wqkv_sb[:, kt, jt * 128:(jt + 1) * 128],
                            hT[:, kt, :, :],
                            start=(kt == 0), stop=(kt == CT - 1))
                    nc.scalar.activation(out=qk[:, jt, :, :], in_=acc,
                                         func=AF.Identity,
                                         bias=qkb_sb[:, jt:jt + 1])
                for ct in range(CT):  # k c-tiles
                    acc = ps.tile([128, 2, T], F32, tag="ps")
                    for kt in range(CT):
                        nc.tensor.matmul(
                            acc,
                            wqkv_sb[:, kt, (CT + ct) * 128:(CT + ct + 1) * 128],
                            hT[:, kt, :, :],
                            start=(kt == 0), stop=(kt == CT - 1))
                    nc.scalar.activation(out=kp[0:64, 0, ct], in_=acc[0:64],
                                         func=AF.Identity,
                                         bias=qkb_sb[0:64, CT + ct:CT + ct + 1])
                    nc.vector.tensor_scalar(
                        out=kp[64:128, 1, ct], in0=acc[64:128],
                        scalar1=qkb_sb[64:128, CT + ct:CT + ct + 1],
                        scalar2=1.0, op0=ALU.add, op1=ALU.mult)
                for e in (0, 1):
                    v = vp.tile([128, TT, H * 65], BF16, tag="v")
                    st["v%d" % e] = v
                    # ones columns at 64 + 65*h for both tt: free AP (tt, h)
                    nc.gpsimd.memset(
                        v.rearrange("p tt (h c) -> p tt h c", c=65)
                        [:, :, :, 64], 1.0)
                    for tt in range(TT):
                        acc = ps.tile([128, C], F32, tag="ps")
                        for kt in range(CT):
                            nc.tensor.matmul(
                                acc,
                                hT[:, kt, e, tt * 128:(tt + 1) * 128],
                                wqkv_sb[:, kt, 2 * C:3 * C],
                                start=(kt == 0), stop=(kt == CT - 1))
                        nc.scalar.copy(
                            out=v[:, tt, :].rearrange(
                                "p (h c) -> p h c", c=65)[:, :, 0:64],
                            in_=acc)

            def s_att_s(e, st, mode="full"):
                """scores + exp + staircase zero for one element, emitted
                per head-pair so PV of pair hp only waits on its own
                exp/zero chain."""
                qk = st["qk"]
                if mode != "s_only":
                    pt0 = ep.tile([128, H, 256], BF16, tag="pt0")
                    pt1 = ep.tile([128, H, 128], BF16, tag="pt1")
                    st["pt%d" % e] = (pt0, pt1)
                kp = st["kp"]
                for hp2 in range(CT):
                    # block0: t2 0:128 vs all t1 (cols 0:256)
                    s0 = ps.tile([128, 2, T], F32, tag="ps")
                    # block1: t2 128:256 vs t1 128:256 only (causal)
                    s1 = ps.tile([128, 2, 128], F32, tag="ps")
                    for i in (0, 1):
                        h = 2 * hp2 + i
                        nc.tensor.matmul(
                            s0[:, i, :],
                            kp[:, h % 2, hp2, e, 0, :],
                            qk[:, hp2, e, :],
                            start=True, stop=True)
                        nc.tensor.matmul(
                            s1[:, i, :],
                            kp[:, h % 2, hp2, e, 1, :],
                            qk[:, hp2, e, 128:256],
                            start=True, stop=True)
                    if mode == "s_only":
                        continue
                    if mode == "mm_only":
                        nc.vector.tensor_copy(
                            out=pt0[:, 2 * hp2:2 * hp2 + 2, :], in_=s0)
                        nc.vector.tensor_copy(
                            out=pt1[:, 2 * hp2:2 * hp2 + 2, :], in_=s1)
                        continue
                    nc.scalar.activation(
                        out=pt0[:, 2 * hp2:2 * hp2 + 2, :], in_=s0,
                        func=AF.Exp, scale=SC)
                    nc.scalar.activation(
                        out=pt1[:, 2 * hp2:2 * hp2 + 2, :], in_=s1,
                        func=AF.Exp, scale=SC)
                    if mode == "no_mask":
                        continue
                    # zero invalid (t1 < t2) staircases of both diagonal
                    # blocks for this head pair: multiply by the 0/1 mask
                    # (bf16 all-sbuf DVE op)
                    pd = pt0[:, 2 * hp2:2 * hp2 + 2, 0:128]
                    nc.vector.tensor_mul(out=pd, in0=pd, in1=mask_bf)
                    pd = pt1[:, 2 * hp2:2 * hp2 + 2, :]
                    nc.vector.tensor_mul(out=pd, in0=pd, in1=mask_bf)

            def s_att_sums(e, st):
                """softmax denominators from the masked pt via base-0 ones
                matmuls, then ln (f32r) - runs concurrently with PV so the
                1/sum broadcast is ready when PV retires."""
                pt0, pt1 = st["pt%d" % e]
                sl = []
                for hp2 in range(CT):
                    smp = ps.tile([1, 2, T], F32, tag="ps")
                    for i in (0, 1):
                        h = 2 * hp2 + i
                        nc.tensor.matmul(
                            smp[:, i, :], ones_bf, pt0[:, h, :],
                            start=True, stop=True, skip_group_check=True)
                        nc.tensor.matmul(
                            smp[:, i, 128:256], ones_bf, pt1[:, h, :],
                            start=False, stop=True, skip_group_check=True)
                    sums_sb = rp.tile([1, 2, T], F32R, tag="sums")
                    with nc.allow_low_precision(
                            reason="softmax 1/sum via f32r ln/exp"):
                        nc.scalar.activation(out=sums_sb, in_=smp,
                                             func=AF.Ln, scale=1.0)
                    sl.append(sums_sb)
                st["sums%d" % e] = sl

            def s_att_pv(e, st, norm=True):
                """ones-augmented PV (sums ride on partition 64), 1/sum via
                ln/exp + PE broadcast, normalize -> attout bf16."""
                v, (pt0, pt1) = st["v%d" % e], st["pt%d" % e]
                attout = ep.tile([128, CT, T], BF16, tag="ao")
                vh = v.rearrange("p tt (h c) -> p tt h c", c=65)
                for hp2 in range(CT):
                    a_ps = ps.tile([65, 2, T], F32, tag="ps")
                    for i in (0, 1):
                        h = 2 * hp2 + i
                        if pv_clean:
                            nc.tensor.matmul(
                                a_ps[:, i, 0:128], vh[:, 0, h, :],
                                pt0[:, h, 0:128], start=True, stop=True)
                            nc.tensor.matmul(
                                a_ps[:, i, 128:256], vh[:, 0, h, :],
                                pt0[:, h, 128:256], start=True, stop=False)
                            nc.tensor.matmul(
                                a_ps[:, i, 128:256], vh[:, 1, h, :],
                                pt1[:, h, :], start=False, stop=True)
                        else:
                            nc.tensor.matmul(
                                a_ps[:, i, :], vh[:, 0, h, :],
                                pt0[:, h, :],
                                start=True, stop=True,
                                skip_group_check=True)
                            nc.tensor.matmul(
                                a_ps[:, i, 128:256], vh[:, 1, h, :],
                                pt1[:, h, :],
                                start=False, stop=True,
                                skip_group_check=True)
                    # 1/sum via exp(-ln(sum)): the sums gather doubles as
                    # the Ln (f32r out feeds the PE broadcast), the Exp
                    # lands broadcast in sbuf. Stays on the pinned ACT table.
                    if not norm:
                        nc.vector.tensor_copy(out=attout[0:64, hp2, :],
                                              in_=a_ps[0:64, 0, :])
                        nc.vector.tensor_copy(out=attout[64:128, hp2, :],
                                              in_=a_ps[0:64, 1, :])
                        continue
                    sumb_ps = ps.tile([64, 2, T], F32, tag="ps")
                    nc.tensor.matmul(
                        sumb_ps, ones_sb[0:1, :], st["sums%d" % e][hp2],
                        start=True, stop=True)
                    recb = rp.tile([64, 2, T], F32, tag="recb")
                    nc.scalar.activation(out=recb, in_=sumb_ps,
                                         func=AF.Exp, scale=-1.0)
                    for i in (0, 1):
                        h = 2 * hp2 + i
                        po = 64 * (h % 2)
                        nc.vector.tensor_mul(
                            out=attout[po:po + 64, hp2, :],
                            in0=a_ps[0:64, i, :], in1=recb[:, i, :])
                st["ao%d" % e] = attout

            def s_proj(e, st):
                attout, x_sb = st["ao%d" % e], st["x%d" % e]
                x1 = x1p.tile([128, TT, C], F32, tag="x1")
                st["x1%d" % e] = x1
                for tt in range(TT):
                    pps = ps.tile([128, C], F32, tag="ps")
                    for ct in range(CT):
                        nc.tensor.matmul(
                            pps,
                            attout[:, ct, tt * 128:(tt + 1) * 128],
                            wproj_sb[:, ct, :],
                            start=(ct == 0), stop=(ct == CT - 1))
                    nc.vector.tensor_add(out=x1[:, tt, :],
                                         in0=x_sb[:, tt, :], in1=pps)
                    nc.gpsimd.tensor_add(out=x1[:, tt, :],
                                         in0=x1[:, tt, :],
                                         in1=vec_sb[:, BPROJ, :])

            def s_ln2(e, st):
                h2 = hp_.tile([128, TT, C], TRD, tag="h2")
                s_ln(st["x1%d" % e], h2, st, "h2%d" % e)

            def s_tr2(p, st):
                """h2 transposes (bf16, proven); pack-copies convert to fp8
                for the DoubleRow FFN. ct 3 is the zero pad (contraction
                512 = 2 DoubleRow k-tile pairs)."""
                if ffn8:
                    h2T = pp.tile([128, CT + 1, 2, T], FP8, tag="h2T")
                    nc.gpsimd.memset(h2T[:, CT], 0.0)
                else:
                    h2T = pp.tile([128, CT, 2, T], BF16, tag="h2T")
                transpose_pair((st["h20"], st["h21"]), h2T)
                st["h2T"] = h2T

            def s_ffn1(p, st):
                h2T = st["h2T"]
                ff = pp.tile([128, FT, 2, T], FP8 if ffn8 else BF16,
                             tag="ff")
                st["ff"] = ff
                for ft in range(FT):
                    acc = ps.tile([128, 2, T], F32, tag="ps")
                    if ffn8:
                        for g in (0, 1):
                            nc.tensor.matmul(
                                acc,
                                w1_sb[:, g, :, ft * 128:(ft + 1) * 128],
                                h2T[:, 2 * g:2 * g + 2, :, :],
                                start=(g == 0), stop=(g == 1),
                                perf_mode=DR)
                    else:
                        for kt in range(CT):
                            nc.tensor.matmul(
                                acc,
                                w1_sb[:, kt, ft * 128:(ft + 1) * 128],
                                h2T[:, kt, :, :],
                                start=(kt == 0), stop=(kt == CT - 1))
                    if ft % 2 == 0:
                        nc.vector.tensor_scalar(
                            out=ff[:, ft, :, :], in0=acc,
                            scalar1=b1_sb[:, ft:ft + 1], scalar2=0.0,
                            op0=ALU.add, op1=ALU.max)
                    else:
                        nc.scalar.activation(out=ff[:, ft, :, :], in_=acc,
                                             func=AF.Relu,
                                             bias=b1_sb[:, ft:ft + 1],
                                             scale=1.0)

            def s_ffn2(e, st, b):
                ff, x1 = st["ff"], st["x1%d" % e]
                ob = outd[b].rearrange("(tt p) c -> p tt c", p=128)
                o_sb = ep.tile([128, TT, C], F32, tag="o")
                for tt in range(TT):
                    fps = ps.tile([128, C], F32, tag="ps")
                    if ffn8:
                        for j in range(FT // 2):
                            nc.tensor.matmul(
                                fps,
                                ff[:, 2 * j:2 * j + 2, e,
                                   tt * 128:(tt + 1) * 128],
                                w2_sb[:, j, :, :],
                                start=(j == 0), stop=(j == FT // 2 - 1),
                                perf_mode=DR)
                        # fp8 scaling: W1 x8, W2 x16 on the host -> psum is
                        # 128x the true ffn; fold 1/128 into the residual add
                        nc.vector.scalar_tensor_tensor(
                            out=o_sb[:, tt, :], in0=fps, scalar=1.0 / 128,
                            in1=x1[:, tt, :],
                            op0=ALU.mult, op1=ALU.add)
                    else:
                        for ft in range(FT):
                            nc.tensor.matmul(
                                fps,
                                ff[:, ft, e, tt * 128:(tt + 1) * 128],
                                w2_sb[:, ft, :],
                                start=(ft == 0), stop=(ft == FT - 1))
                        nc.vector.tensor_add(out=o_sb[:, tt, :],
                                             in0=x1[:, tt, :], in1=fps)
                    nc.gpsimd.tensor_add(out=o_sb[:, tt, :],
                                         in0=o_sb[:, tt, :],
                                         in1=vec_sb[:, B2, :])
                nc.sync.dma_start(out=ob, in_=o_sb)

            def s_front(p, states):
                """x loads + LN1 for both elements of pair p."""
                st = states[p] = {}
                for e in (0, 1):
                    est = {}
                    s_ln1(2 * p + e, est)
                    st["x%d" % e] = est["x"]
                    st["h%d" % e] = est["h"]

            def s_mid(p, st):
                s_tr1(p, st)
                s_qkv(p, st)

            def emit_dbg(st):
                """DMA out pair-0/elem-0 intermediates for HW-vs-sim diff."""
                for nm, shp, src in [
                        ("dh", [128, TT, C], st["h0"]),
                        ("dhT", [128, CT, 2, T], st["hT"]),
                        ("dqk", [128, CT, 2, T], st["qk"]),
                        ("dkp", [128, 2, CT, 2, 2, 128], st["kp"]),
                        ("dpt0", [128, H, 256], st["pt0"][0]),
                        ("dpt1", [128, H, 128], st["pt0"][1]),
                        ("dao", [128, CT, T], st["ao0"]),
                        ("dv", [128, TT, H * 65], st["v0"]),
                ]:
                    t = nc.dram_tensor(nm, shp, src.dtype,
                                       kind="ExternalOutput")
                    nc.sync.dma_start(out=t[...], in_=src)

            def emit_all():
                states = {}
                s_front(0, states)
                s_mid(0, states[0])
                if NP > 1:
                    s_front(1, states)
                for p in range(NP):
                    st = states[p]
                    prev = states.get(p - 1)
                    if "attn" in skip:
                        st["ao0"] = st["hT"][:, :, 0, :]
                        st["ao1"] = st["hT"][:, :, 1, :]
                    elif "pv" in skip:
                        m = [s for s in skip if s != "pv"]
                        m = m[0] if m else "full"
                        s_att_s(0, st, mode=m)
                        s_att_s(1, st, mode=m)
                        st["ao0"] = st["hT"][:, :, 0, :]
                        st["ao1"] = st["hT"][:, :, 1, :]
                    else:
                        nrm = "nonorm" not in skip
                        s_att_s(0, st)
                        s_att_s(1, st)
                        s_att_sums(0, st)
                        s_att_pv(0, st, norm=nrm)
                        s_att_sums(1, st)
                        s_att_pv(1, st, norm=nrm)
                    if dbg and p == 0:
                        emit_dbg(st)
                    s_proj(0, st)
                    s_proj(1, st)
                    if p + 1 < NP:
                        s_tr1(p + 1, states[p + 1])
                    if prev is not None:
                        # previous pair's FFN2: PE filler while this pair's
                        # LN2 chains run on the vector engines
                        s_ffn2(0, prev, 2 * (p - 1))
                    if p + 1 < NP:
                        s_qkv(p + 1, states[p + 1])
                    s_ln2(0, st)
                    s_ln2(1, st)
                    if prev is not None:
                        s_ffn2(1, prev, 2 * (p - 1) + 1)
                        del states[p - 1]
                    s_tr2(p, st)
                    s_ffn1(p, st)
                    if p + 2 < NP:
                        s_front(p + 2, states)
                last = states[NP - 1]
                s_ffn2(0, last, 2 * (NP - 1))
                s_ffn2(1, last, 2 * (NP - 1) + 1)
                del states[NP - 1]

            if loop_reps > 1:
                with tc.For_i(0, loop_reps, 1):
                    for _ in range(reps):
                        emit_all()
            else:
                for _ in range(reps):
                    emit_all()

    nc.compile()
    return nc


def _prep_maps(x, Wqkv, Wproj, bproj, W1, b1, W2, b2, g1, be1, g2, be2,
               nb=BL, ffn8=False):
    import ml_dtypes
    f32 = np.float32
    f64 = np.float64
    bf16 = ml_dtypes.bfloat16
    f8 = ml_dtypes.float8_e4m3
    Wqkv, Wproj = np.asarray(Wqkv, f64), np.asarray(Wproj, f64)
    W1, W2 = np.asarray(W1, f64), np.asarray(W2, f64)
    g1, be1 = np.asarray(g1, f64), np.asarray(be1, f64)
    g2, be2 = np.asarray(g2, f64), np.asarray(be2, f64)
    bproj, b1, b2 = (np.asarray(bproj, f64), np.asarray(b1, f64),
                     np.asarray(b2, f64))
    # fold LN gains into the consuming weights, LN betas into biases:
    #   h = z*g + be  =>  h @ W.T = z @ (W*g).T + (W @ be)
    Wqkv_g = Wqkv * g1[None, :]
    b_qkv = Wqkv @ be1                       # [3C]; q,k parts applied at copy
    bproj_eff = bproj + Wproj @ b_qkv[2 * C:]  # v bias folded via softmax sum=1
    W1_g = W1 * g2[None, :]
    b1_eff = b1 + W1 @ be2
    bcast = lambda v: np.ascontiguousarray(
        np.broadcast_to(np.asarray(v, f32), (128, C)))
    vecs = np.stack([bcast(bproj_eff), bcast(b2)])  # [2,128,C]
    if ffn8:
        # DoubleRow fp8: W1 x8, W2 x16 to sit in e4m3's normal range; the
        # 1/128 is folded into the FFN2 residual add. k-tile-pair layouts:
        # cin = g*256 + i*128 + p (W1, zero-padded 384->512),
        # f = j*256 + i*128 + p (W2).
        W1p = np.zeros((512, 4 * C), f64)
        W1p[:C] = W1_g.T * 8.0
        w1x = np.ascontiguousarray(W1p.reshape(2, 2, 128, 4 * C).astype(f8))
        w2x = np.ascontiguousarray(
            (W2.T * 16.0).reshape(FT // 2, 2, 128, C).astype(f8))
        b1_host = (b1_eff * 8.0).astype(f32)
    else:
        w1x = np.ascontiguousarray(W1_g.astype(bf16).T).reshape(
            CT, 128, 4 * C)
        w2x = np.ascontiguousarray(W2.astype(bf16).T).reshape(FT, 128, C)
        b1_host = b1_eff.astype(f32)
    shared = {
        "wqkv": np.ascontiguousarray(Wqkv_g.astype(bf16).T).reshape(
            CT, 128, 3 * C),
        "wproj": np.ascontiguousarray(Wproj.astype(bf16).T).reshape(
            CT, 128, C),
        "w1": w1x,
        "w2": w2x,
        "b1": np.ascontiguousarray(b1_host.reshape(FT, 128)),
        "qkb": np.ascontiguousarray(b_qkv[:2 * C].astype(f32).reshape(
            2 * CT, 128)),
        "vecs": vecs,
        "ones": np.ones((128, 64), f32),
    }
    x = np.asarray(x, f32)
    return [dict(shared, x=np.ascontiguousarray(x[i * nb:(i + 1) * nb]))
            for i in range(NCORES)]


def run(inputs, reps=1, trace=False, nb=BL):
    from concourse import bass_utils
    key = ("nc", reps, nb)
    if key not in _cache:
        _cache[key] = build(reps, nb)
    nc = _cache[key]
    in_maps = _prep_maps(**inputs, nb=nb)
    res = bass_utils.run_bass_kernel_spmd(
        nc, in_maps, core_ids=list(range(NCORES)), trace=trace)
    out = np.concatenate([res.results[i]["out"] for i in range(NCORES)], axis=0)
    return out, res


def kernel(**inputs):
    out, _ = run(inputs)
    return out


# ---------- cached jitted runner for benchmarking (execute-only calls) ----------
def get_runner(reps=1, nb=BL, loop_reps=1, skip=()):
    """Returns (call, put) where put(in_maps) -> device args and call(args)
    executes the prebuilt NEFF on 8 cores, returning jax output arrays."""
    import jax
    import numpy as _np
    from jax.experimental.shard_map import shard_map
    from jax.sharding import Mesh, PartitionSpec, NamedSharding
    from concourse import bass2jax as B2J
    import concourse.mybir as mybir

    key = ("runner", reps, nb, loop_reps, tuple(skip))
    if key in _cache:
        return _cache[key]
    nckey = ("nc", reps, nb, loop_reps, tuple(skip))
    if nckey not in _cache:
        _cache[nckey] = build(reps, nb, loop_reps=loop_reps, skip=skip)
    nc = _cache[nckey]

    B2J.install_neuronx_cc_hook()
    part_name = (nc.partition_id_tensor.name if nc.partition_id_tensor
                 else None)
    in_names, out_names, out_avals, zero_outs = [], [], [], []
    for alloc in nc.m.functions[0].allocations:
        if not isinstance(alloc, mybir.MemoryLocationSet):
            continue
        name = alloc.memorylocations[0].name
        if alloc.kind == "ExternalInput":
            if name != part_name:
                in_names.append(name)
        elif alloc.kind == "ExternalOutput":
            out_names.append(name)
            shape = tuple(alloc.tensor_shape)
            dtype = mybir.dt.np(alloc.dtype)
            out_avals.append(jax.core.ShapedArray(shape, dtype))
            zero_outs.append(_np.zeros(shape, dtype))
    n_params = len(in_names)
    all_names = in_names + out_names
    if part_name is not None:
        all_names = all_names + [part_name]

    def _body(*args):
        operands = list(args)
        if part_name is not None:
            operands.append(B2J.partition_id_tensor())
        outs = B2J._bass_exec_p.bind(
            *operands,
            out_avals=tuple(out_avals),
            in_names=tuple(all_names),
            out_names=tuple(out_names),
            lowering_input_output_aliases=(),
            sim_require_finite=True,
            sim_require_nnan=True,
            nc=nc,
        )
        return tuple(outs)

    devices = jax.devices()[:NCORES]
    mesh = Mesh(_np.asarray(devices), ("core",))
    spec = PartitionSpec("core")
    n_outs = len(out_names)
    sharded = jax.jit(
        shard_map(_body, mesh=mesh, in_specs=(spec,) * (n_params + n_outs),
                  out_specs=(spec,) * n_outs, check_rep=False),
        keep_unused=True)
    sharding = NamedSharding(mesh, spec)

    def put(in_maps):
        args = []
        for i, name in enumerate(in_names):
            cat = _np.concatenate([_np.asarray(m[name]) for m in in_maps], 0)
            args.append(jax.device_put(cat, sharding))
        for z in zero_outs:
            cat = _np.zeros((NCORES * z.shape[0], *z.shape[1:]), z.dtype)
            args.append(jax.device_put(cat, sharding))
        return args

    def call(args):
        outs = sharded(*args)
        jax.block_until_ready(outs)
        return outs

    _cache[key] = (call, put)
    return call, put
